# revision 10
# baseline (speedup 1.0000x reference)
"""Trainium2 Bass kernel v6: FPS (npoint=1024) + ball-query(r=0.5, nsample=1)
over B=4 clouds of N=16384 points.

Sharding: batch b runs on cores 2b and 2b+1 (same FPS, duplicated); the ball
query is split by DATA between the pair — even core scans points [0,8192),
odd core [8192,16384). Host combines with elementwise min.

The FPS step is latency-bound: ~2.0us of dependent work across four
engines per step vs ~1.2us of DVE busy time. The step chain:
  rowmax (DVE) -> partition_all_reduce max on Pool (bc: global max bcast)
  -> um = (pmax==bc) partition one-hot (DVE) -> PE row extraction
  (matmul(lhsT=dist, rhs=um) pulls the winner's dist row into PSUM;
  3 more pull the negated coord rows) -> wf/cmask winner-column isolate
  (DVE, split in two ops: walrus allows one non-scalar PSUM input) ->
  partition_all_reduce add on Pool (cbc: -coords bcast, lands in SBUF so
  ACT can use it as a bias) -> ACT Square(p3*1+cbc) for y,z fused, DVE
  dxx+sqx for x (balance: 3 ACT squares serialize and lose ~0.5us/step)
  -> s2/s3/min on DVE in the reference's exact add order.
All selector products are one-hot so extraction is bit-exact; the ACT
square-with-bias fusion and both Pool all-reduces are HW-verified
bit-exact (0/4096 index mismatches over multiple seeds).

The ball query (per 128-centroid tile: stat build, then 32 dps matmuls
(W=256 point blocks; measured faster than 64x128 or 16x512) ->
ACT PSUM->SBUF copy -> DVE candb/reduce) rides in the chain's stall
windows. Emission goes through a unit graph + greedy model, but note the
Tile rust scheduler REORDERS per-engine by readiness (emission order and
bass_priority hints measurably do not matter; bass_wait_until_ts floors
do but measured slower). What does matter: the DAG, op costs, engine
assignment, and pool buffer counts.

Measured (CoreSim cost model, per core): v2 baseline 2.814ms ->
v3 (Pool PARs + ACT bias fusion) 2.307 -> v4 (PE row extraction) 2.220 ->
v6 (ACT dps copies) 2.194 -> W=256 2.186ms. Dead ends: tensor_tensor_reduce (neuronxcc
crash in every variant), Pool tensor_tensor/reduce (rejected), 3-chain
SMT (redundant chains can't shorten the real chain's latency), tail
half-split (per-op init overhead), all-ACT squares, PE-based bc.
"""

import os
import numpy as np

B, N, S, P, F = 4, 16384, 1024, 128, 128
NCORES = 8
HALF = N // 2
R2 = 0.25
BIG = 1e10
W = 256
NBLK = HALF // W
NSTEP = S
SEM = 100.0
CHAIN_BIAS = 64.0  # prefer chain units over fillers within this slack

_cache = {}


class Unit:
    __slots__ = ("eng", "cost", "fn", "deps", "done")

    def __init__(self, eng, cost, fn, deps):
        self.eng = eng
        self.cost = cost
        self.fn = fn
        self.deps = deps
        self.done = None


def _build_program():
    import concourse.mybir as mybir
    from concourse import bacc, bass_isa
    from concourse.tile import TileContext

    f32 = mybir.dt.float32
    i32 = mybir.dt.int32
    X = mybir.AxisListType
    op = mybir.AluOpType
    Square = mybir.ActivationFunctionType.Square

    nc = bacc.Bacc(None, target_bir_lowering=False)

    P3d = nc.declare_dram_parameter("p3", [P, 3, F], f32, isOutput=False)
    NP3d = nc.declare_dram_parameter("negp3", [P, 3, F], f32, isOutput=False)
    MOVd = nc.declare_dram_parameter("mov", [5, HALF], f32, isOutput=False)
    RNd = nc.declare_dram_parameter("revn", [P, HALF], f32, isOutput=False)
    C1d = nc.declare_dram_parameter("colone", [P, 1], f32, isOutput=False)
    SAd = nc.declare_dram_parameter("sel_a", [3, 5], f32, isOutput=False)
    SBd = nc.declare_dram_parameter("sel_b", [3, 5], f32, isOutput=False)
    E4d = nc.declare_dram_parameter("e4", [1, 5], f32, isOutput=False)
    O1d = nc.declare_dram_parameter("one1", [1, P], f32, isOutput=False)
    OUTd = nc.declare_dram_parameter("oidx", [8, P], i32, isOutput=True)

    DVE, ACT, PE, POOL = "DVE", "ACT", "PE", "POOL"

    with TileContext(nc) as tc:
        with (
            tc.tile_pool(name="const", bufs=1) as cpool,
            tc.tile_pool(name="work", bufs=2) as wpool,
            tc.tile_pool(name="ps_small", bufs=1, space="PSUM") as spool,
            tc.tile_pool(name="ps_rx", bufs=2, space="PSUM") as rxpool,
            tc.tile_pool(name="ps_cw", bufs=2, space="PSUM") as cwpool,
            tc.tile_pool(name="ps_ball", bufs=2, space="PSUM") as bpool,
        ):
            p3 = cpool.tile([P, 3, F], f32, tag="p3")
            nc.sync.dma_start(out=p3[:, :, :], in_=P3d[:, :, :])
            negp3 = cpool.tile([P, 3, F], f32, tag="negp3")
            nc.sync.dma_start(out=negp3[:, :, :], in_=NP3d[:, :, :])
            colone = cpool.tile([P, 1], f32, tag="colone")
            nc.sync.dma_start(out=colone[:, :], in_=C1d[:, :])
            sel_a = cpool.tile([3, 5], f32, tag="sel_a")
            nc.sync.dma_start(out=sel_a[:, :], in_=SAd[:, :])
            sel_b = cpool.tile([3, 5], f32, tag="sel_b")
            nc.sync.dma_start(out=sel_b[:, :], in_=SBd[:, :])
            e4 = cpool.tile([1, 5], f32, tag="e4")
            nc.sync.dma_start(out=e4[:, :], in_=E4d[:, :])
            one1 = cpool.tile([1, P], f32, tag="one1")
            nc.sync.dma_start(out=one1[:, :], in_=O1d[:, :])
            mov = cpool.tile([5, HALF], f32, tag="mov")
            nc.sync.dma_start(out=mov[:, :], in_=MOVd[:, :])
            revn = cpool.tile([P, HALF], f32, tag="revn")
            nc.sync.dma_start(out=revn[:, :], in_=RNd[:, :])

            dist = cpool.tile([P, F], f32, tag="dist")
            nc.vector.memset(dist[:, :], BIG)
            pmax = cpool.tile([P, 1], f32, tag="pmax")
            nc.vector.tensor_reduce(pmax[:, :], p3[:, 0, :], axis=X.X,
                                    op=op.max)

            CH, FPE, FACT, FDVE = 0, 1, 2, 3
            streams = [[] for _ in range(4)]
            is_chain = [True, False, False, False]
            cwaccs = {}
            st = {}

            def unit(stream, eng, cost, deps, fn):
                u = Unit(eng, cost, fn, [d for d in deps if d is not None])
                streams[stream].append(u)
                return u

            def fps_units(s):
                masksrc = p3[:, 0, :] if s == 0 else dist[:, :]
                prev = st.get("pmax_u")

                bc = wpool.tile([P, 1], f32, tag="bc", name=f"bc_{s}")

                def e_bc():
                    nc.gpsimd.partition_all_reduce(
                        out_ap=bc[:, :], in_ap=pmax[:, :], channels=P,
                        reduce_op=bass_isa.ReduceOp.max)
                u_bc = unit(CH, POOL, 4, [prev], e_bc)

                um = wpool.tile([P, 1], f32, tag="um", name=f"um_{s}")

                def e_um():
                    nc.vector.tensor_scalar(
                        out=um[:, :], in0=pmax[:, :], scalar1=bc[:, :],
                        scalar2=None, op0=op.is_equal)
                u_um = unit(CH, DVE, 10, [u_bc], e_um)

                rx = rxpool.tile([P, 4], f32, tag="rx", name=f"rx_{s}")

                def e_rx():
                    nc.tensor.matmul(rx[:, 0:1], lhsT=masksrc, rhs=um[:, :],
                                     start=True, stop=True)
                    for c in range(3):
                        nc.tensor.matmul(rx[:, c + 1:c + 2],
                                         lhsT=negp3[:, c, :], rhs=um[:, :],
                                         start=True, stop=True)
                u_rx = unit(CH, PE, 10, [u_um], e_rx)

                cmask = wpool.tile([P, 3], f32, tag="cmask",
                                   name=f"cmask_{s}")

                def e_cmask():
                    wf = wpool.tile([P, 1], f32, tag="wf", name=f"wf_{s}")
                    nc.vector.tensor_scalar(
                        out=wf[:, :], in0=rx[:, 0:1], scalar1=bc[:, :],
                        scalar2=None, op0=op.is_equal)
                    nc.vector.tensor_scalar(
                        out=cmask[:, :], in0=rx[:, 1:4], scalar1=wf[:, :],
                        scalar2=None, op0=op.mult)
                u_cmask = unit(CH, DVE, 140, [u_rx], e_cmask)

                cbc = wpool.tile([P, 3], f32, tag="cbc", name=f"cbc_{s}")

                def e_cbc():
                    nc.gpsimd.partition_all_reduce(
                        out_ap=cbc[:, :], in_ap=cmask[:, :], channels=P,
                        reduce_op=bass_isa.ReduceOp.add)
                u_cbc = unit(CH, POOL, 4, [u_cmask], e_cbc)

                t = s // P
                if s % P == 0:
                    cwaccs[t] = cwpool.tile([3, P], f32, tag="cwacc",
                                            name=f"cwacc{t}")
                cwacc = cwaccs[t]

                def e_cw():
                    nc.tensor.matmul(cwacc[:, s % P:s % P + 1],
                                     lhsT=cmask[:, :], rhs=colone[:, :],
                                     start=True, stop=True)
                st[("cw", s)] = unit(CH, PE, 3, [u_cmask], e_cw)

                if s >= S - 1:
                    st["pmax_u"] = st[("cw", s)]
                    return

                sqy = wpool.tile([P, F], f32, tag="sqy", name=f"sqy_{s}")
                sqz = wpool.tile([P, F], f32, tag="sqz", name=f"sqz_{s}")

                def e_sqy():
                    nc.scalar.activation(out=sqy[:, :], in_=p3[:, 1, :],
                                         func=Square, bias=cbc[:, 1:2],
                                         scale=1.0)
                u_sqy = unit(CH, ACT, 292, [u_cbc], e_sqy)

                def e_sqz():
                    nc.scalar.activation(out=sqz[:, :], in_=p3[:, 2, :],
                                         func=Square, bias=cbc[:, 2:3],
                                         scale=1.0)
                u_sqz = unit(CH, ACT, 292, [u_cbc], e_sqz)

                sqx = wpool.tile([P, F], f32, tag="sqx", name=f"sqx_{s}")

                def e_sqx():
                    dxx = wpool.tile([P, F], f32, tag="dxx", name=f"dxx_{s}")
                    nc.vector.tensor_scalar(
                        out=dxx[:, :], in0=p3[:, 0, :], scalar1=cbc[:, 0:1],
                        scalar2=None, op0=op.add)
                    nc.vector.tensor_tensor(out=sqx[:, :], in0=dxx[:, :],
                                            in1=dxx[:, :], op=op.mult)
                u_sqx = unit(CH, DVE, 321, [u_cbc], e_sqx)

                s2 = wpool.tile([P, F], f32, tag="s2", name=f"s2_{s}")

                def e_s2():
                    nc.vector.tensor_tensor(out=s2[:, :], in0=sqx[:, :],
                                            in1=sqy[:, :], op=op.add)
                u_s2 = unit(CH, DVE, 194, [u_sqx, u_sqy], e_s2)

                s3 = wpool.tile([P, F], f32, tag="s3", name=f"s3_{s}")

                def e_s3():
                    nc.vector.tensor_tensor(out=s3[:, :], in0=s2[:, :],
                                            in1=sqz[:, :], op=op.add)
                u_s3 = unit(CH, DVE, 194, [u_s2, u_sqz], e_s3)

                def e_min():
                    nc.vector.tensor_tensor(out=dist[:, :], in0=dist[:, :],
                                            in1=s3[:, :], op=op.min)
                u_min = unit(CH, DVE, 194, [u_s3], e_min)

                def e_pmax():
                    nc.vector.tensor_reduce(pmax[:, :], dist[:, :],
                                            axis=X.X, op=op.max)
                st["pmax_u"] = st[("pmax", s)] = unit(CH, DVE, 194,
                                                      [u_min], e_pmax)

            STEP_MS = 1.99e-3  # est step period for scheduler pacing (ms)
            USE_WAITS = False  # measured slower in every variant
            WAIT_CAP = 960     # no floors past this step (tail stays dep-driven)
            BALL_DEPRIORITIZE = True

            def waited(fn, step_eq):
                # Ball-query emissions: optionally floor the Tile
                # scheduler's start time (USE_WAITS), and push their
                # bass_priority far below every chain instruction so a
                # backlogged ball op never wins an engine over a ready
                # chain op (the Tile scheduler breaks ready-ties by
                # priority = emission index).
                use_wait = USE_WAITS and step_eq is not None \
                    and step_eq <= WAIT_CAP
                ms = (step_eq if step_eq is not None else 0) * STEP_MS

                def wrapped():
                    from contextlib import ExitStack
                    with ExitStack() as ctx:
                        if BALL_DEPRIORITIZE:
                            ctx.enter_context(
                                tc.high_priority(offset=-1000000))
                        if use_wait:
                            ctx.enter_context(tc.tile_wait_until(ms))
                        fn()
                return wrapped

            def ball_units(t):
                dep_cw = st[("cw", t * P + P - 1)]
                cwacc = cwaccs.pop(t)
                cwsb = wpool.tile([3, P], f32, tag="cwsb", name=f"cwsb{t}")

                def e_cwsb():
                    nc.scalar.copy(out=cwsb[:, :], in_=cwacc[:, :])
                u_cwsb = unit(FACT, ACT, 150, [dep_cw],
                              waited(e_cwsb, None))

                sqcw = wpool.tile([3, P], f32, tag="sqcw", name=f"sqcw{t}")

                def e_sqcw():
                    nc.vector.tensor_tensor(out=sqcw[:, :], in0=cwsb[:, :],
                                            in1=cwsb[:, :], op=op.mult)
                u_sqcw = unit(FDVE, DVE, 140, [u_cwsb],
                              waited(e_sqcw, None))

                statp = spool.tile([5, P], f32, tag="statp", name=f"statp{t}")

                def e_statp():
                    nc.tensor.matmul(statp[:, :], lhsT=sel_a[:, :],
                                     rhs=cwsb[:, :], start=True, stop=False)
                    nc.tensor.matmul(statp[:, :], lhsT=sel_b[:, :],
                                     rhs=sqcw[:, :], start=False, stop=False)
                    nc.tensor.matmul(statp[:, :], lhsT=e4[:, :],
                                     rhs=one1[:, :], start=False, stop=True)
                u_statp = unit(FPE, PE, 60, [u_cwsb, u_sqcw],
                               waited(e_statp, None))

                stat = wpool.tile([5, P], f32, tag="stat", name=f"stat{t}")

                def e_stat():
                    nc.scalar.copy(out=stat[:, :], in_=statp[:, :])
                u_stat = unit(FACT, ACT, 150, [u_statp],
                              waited(e_stat, None))

                bests = wpool.tile([P, NBLK], f32, tag="bests",
                                   name=f"bests{t}")
                u_reds = []
                for blk in range(NBLK):
                    dps = bpool.tile([P, W], f32, tag="dps",
                                     name=f"dps{t}_{blk}")

                    def mk_dps(dps, blk):
                        def e_dps():
                            nc.tensor.matmul(
                                dps[:, :], lhsT=stat[:, :],
                                rhs=mov[:, blk * W:(blk + 1) * W],
                                start=True, stop=True)
                        return e_dps
                    cps = st.setdefault("cps", [])
                    # pace: block k of tile t may not schedule before the
                    # chain reaches step 128t+127+2k (spreads the ball work
                    # over the next tile's 128 steps instead of flooding)
                    anchor = st.get(("pmax", min(t * P + P - 1 + 2 * blk,
                                                 S - 2)))
                    a_k = t * P + P + 5 + 2 * blk
                    u_dps = unit(FPE, PE, 330,
                                 [u_stat, anchor,
                                  cps[-2] if len(cps) >= 2 else None],
                                 waited(mk_dps(dps, blk), a_k))

                    dpss = wpool.tile([P, W], f32, tag="dpss",
                                      name=f"dpss{t}_{blk}")

                    def mk_cpy(dps, dpss):
                        def e_cpy():
                            nc.scalar.copy(out=dpss[:, :], in_=dps[:, :])
                        return e_cpy
                    cands = st.setdefault("cands", [])
                    u_cpy = unit(FACT, ACT, 420,
                                 [u_dps,
                                  cands[-2] if len(cands) >= 2 else None],
                                 waited(mk_cpy(dps, dpss), None))
                    cps.append(u_cpy)

                    candb = wpool.tile([P, W], f32, tag="candb",
                                       name=f"candb{t}_{blk}")

                    def mk_cand(dpss, candb, blk):
                        def e_cand():
                            nc.vector.scalar_tensor_tensor(
                                out=candb[:, :], in0=dpss[:, :], scalar=R2,
                                in1=revn[:, blk * W:(blk + 1) * W],
                                op0=op.is_le, op1=op.mult)
                        return e_cand
                    u_cand = unit(FDVE, DVE, 330, [u_cpy],
                                  waited(mk_cand(dpss, candb, blk),
                                         None))
                    cands.append(u_cand)

                    def mk_red(candb, blk):
                        def e_red():
                            nc.vector.tensor_reduce(
                                bests[:, blk:blk + 1], candb[:, :],
                                axis=X.X, op=op.max)
                        return e_red
                    u_reds.append(unit(FDVE, DVE, 330, [u_cand],
                                       waited(mk_red(candb, blk),
                                              None)))

                def e_out():
                    brev = wpool.tile([P, 1], f32, tag="brev",
                                      name=f"brev{t}")
                    nc.vector.tensor_reduce(brev[:, :], bests[:, :],
                                            axis=X.X, op=op.max)
                    oidxf = wpool.tile([P, 1], f32, tag="oidxf",
                                       name=f"oidxf{t}")
                    nc.vector.tensor_scalar(out=oidxf[:, :], in0=brev[:, :],
                                            scalar1=-1.0, scalar2=float(N),
                                            op0=op.mult, op1=op.add)
                    oidxi = wpool.tile([P, 1], i32, tag="oidxi",
                                       name=f"oidxi{t}")
                    nc.vector.tensor_copy(out=oidxi[:, :], in_=oidxf[:, :])
                    nc.sync.dma_start(out=OUTd[t:t + 1, :],
                                      in_=oidxi[:, 0:1])
                unit(FDVE, DVE, 120, [u_reds[-1]],
                     waited(e_out, None))

            for s in range(NSTEP):
                fps_units(s)
            for t in range(NSTEP // P):
                ball_units(t)

            # ---------------- greedy list scheduler ----------------
            eng_time = {DVE: 0.0, ACT: 0.0, PE: 0.0, POOL: 0.0}
            heads = [0] * len(streams)
            n_left = sum(len(x) for x in streams)
            ch = streams[CH]

            def unit_ready(u):
                ready = 0.0
                for d in u.deps:
                    if d.done is None:
                        return None
                    ready = max(ready, d.done +
                                (SEM if d.eng != u.eng else 0.0))
                return ready

            def chain_next_ready(eng):
                # est ready time of the chain's next unscheduled unit on eng
                # (None if unknown — blocked behind unscheduled deps)
                for i in range(heads[CH], min(heads[CH] + 12, len(ch))):
                    u = ch[i]
                    if u.done is not None:
                        continue
                    if u.eng == eng:
                        return unit_ready(u)
                    if u.done is None and unit_ready(u) is None:
                        return None
                return None

            MARGIN = 32.0
            while n_left:
                best, best_key = None, None
                for si, stream in enumerate(streams):
                    hi = heads[si]
                    if hi >= len(stream):
                        continue
                    u = stream[hi]
                    ready = unit_ready(u)
                    if ready is None:
                        continue
                    start = max(eng_time[u.eng], ready)
                    if not is_chain[si]:
                        # don't let a filler delay the chain's next op on
                        # this engine (in-order engines!)
                        cr = chain_next_ready(u.eng)
                        if cr is not None and start + u.cost + MARGIN > cr \
                                and cr >= eng_time[u.eng]:
                            continue
                        key = start + CHAIN_BIAS
                    else:
                        key = start
                    if best is None or key < best_key:
                        best, best_key = (si, u, start), key
                if best is None:
                    # all fillers vetoed — force the chain head (stall) or,
                    # if chain is exhausted, the earliest filler
                    cand = []
                    for si, stream in enumerate(streams):
                        hi = heads[si]
                        if hi >= len(stream):
                            continue
                        u = stream[hi]
                        ready = unit_ready(u)
                        if ready is None:
                            continue
                        cand.append((max(eng_time[u.eng], ready), si, u))
                    if not cand:
                        raise RuntimeError("scheduler deadlock")
                    start, si, u = min(cand, key=lambda x: x[0])
                    best = (si, u, start)
                si, u, start = best
                u.fn()
                u.done = start + u.cost
                eng_time[u.eng] = u.done
                heads[si] += 1
                n_left -= 1

    nc.finalize()
    return nc


def _const_inputs():
    """Input-independent device tensors, built once per process."""
    if "consts" in _cache:
        return _cache["consts"]
    gi = np.arange(N, dtype=np.float32)
    rev = (np.float32(N) - gi).astype(np.float32)
    sel_a = np.zeros((3, 5), dtype=np.float32)
    sel_a[0, 0] = sel_a[1, 1] = sel_a[2, 2] = 2.0
    sel_b = np.zeros((3, 5), dtype=np.float32)
    sel_b[:, 3] = 1.0
    e4 = np.zeros((1, 5), dtype=np.float32)
    e4[0, 4] = 1.0
    consts = {
        "colone": np.ones((P, 1), dtype=np.float32),
        "one1": np.ones((1, P), dtype=np.float32),
        "sel_a": sel_a, "sel_b": sel_b, "e4": e4,
    }
    revn = [np.ascontiguousarray(
        np.broadcast_to(rev[h * HALF:(h + 1) * HALF], (P, HALF)))
        for h in range(2)]
    _cache["consts"] = (consts, revn)
    return _cache["consts"]


def _prep_inputs(xyz):
    """Per-core device input tensors (numpy f32, exact reference layouts)."""
    xyz = np.asarray(xyz, dtype=np.float32)
    consts, revn = _const_inputs()
    maps = []
    for b in range(B):
        x, y, z = xyz[b, 0], xyz[b, 1], xyz[b, 2]
        p3 = np.stack([x.reshape(P, F), y.reshape(P, F), z.reshape(P, F)],
                      axis=1).astype(np.float32)
        pn2 = ((x * x + y * y) + z * z).astype(np.float32)
        movt = np.stack([x, y, z, np.ones(N, np.float32), pn2]).astype(
            np.float32)
        for h in range(2):
            sl = slice(h * HALF, (h + 1) * HALF)
            maps.append({
                "p3": p3, "negp3": -p3,
                "mov": movt[:, sl].copy(),
                "revn": revn[h],
                **consts,
            })
    return maps


def kernel(xyz, cls_label, npoint):
    from concourse.bass_utils import run_bass_kernel_spmd

    assert int(npoint) == S
    if "nc" not in _cache:
        _cache["nc"] = _build_program()
    nc = _cache["nc"]

    in_maps = _prep_inputs(xyz)
    trace = bool(int(os.environ.get("KERNEL_TRACE", "0")))
    res = run_bass_kernel_spmd(nc, in_maps, list(range(NCORES)), trace=trace)
    _cache["last_exec_time_ns"] = res.exec_time_ns

    out = np.zeros((B, S, 1), dtype=np.int32)
    for b in range(B):
        even = np.asarray(res.results[2 * b]["oidx"]).reshape(S)
        odd = np.asarray(res.results[2 * b + 1]["oidx"]).reshape(S)
        out[b, :, 0] = np.minimum(even, odd)
    return out
